# revision 24
# baseline (speedup 1.0000x reference)
"""Causal multi-head self-attention on 8 Trainium2 NeuronCores.

Problem: X[4, 2048, 1024] fp32, W_Q/W_K/W_V/W_O [1024, 1024] fp32,
16 heads x 64 dims, causal softmax attention + output projection.

Sharding: core c handles batch b = c//2 and head-group g = c%2
(heads g*8..g*8+8, i.e. 512 of the 1024 channels).  Each core computes
its 8 heads' Q/K/V projections, causal attention, and a partial output
projection against W_O[:, g*512:(g+1)*512]; the host sums the two
partial outputs per batch (the "all-reduce after W_O" step).

Device kernel layout notes (v3 = v2 schedule + fp8 DoubleRow projections):
 - Q/K projections (all token tiles) and the V projection (token tiles
   1-3) run as fp8e4m3 DoubleRow matmuls: contraction 256 per matmul
   (128 partitions x 2 packed), halving the matmul count.  Inputs are
   host-quantized; accumulation stays fp32 in PSUM, and Q/K/V are
   written back as bf16, so only the *inputs* of these GEMMs are fp8.
   V for tokens 0-511 stays bf16 (early queries put ~all softmax weight
   on the first keys, so those V rows need bf16 accuracy; later keys
   are averaged over >=512 positions and fp8 noise washes out).
 - Attention (scores, exp, PV) is bf16 throughout: with a 64/128-wide
   contraction DoubleRow cannot reduce the streamed row count, so fp8
   would win nothing there.
 - The two per-chunk score matmuls use disjoint contraction partitions
   (heads at 0-63 / 64-127) and run concurrently on the PE array via
   row tiling.
 - Attention runs in "half-groups" of ONE 128-key chunk covering both
   heads, double-buffered score PSUM, PV software-pipelined two
   half-groups behind, projection/output-projection thunks interleaved
   between half-groups (see v2 notes in git history).
   PSUM budget: 4 (scores) + 2 (ops) + 2 (proj) = 8 banks.
 - All host-side tensors are pre-arranged so every DMA is >=1KB
   contiguous per partition; prologue DMAs are spread across the
   sync/vector/scalar/gpsimd queues so the first projection's operands
   (xt tile 0 + W_V) don't serialize behind each other.
 - Softmax skips the max-subtraction (scores are bounded ~|1.9| after
   the 1/8 scale); causal masking multiplies the diagonal score chunks
   by a 0/1 mask after exp; V carries a ones column so PSUM rows 64
   hold the softmax row-sums (normalization = reciprocal + multiply).
"""

import sys

if "/opt/trn_rl_repo" not in sys.path:
    sys.path.insert(0, "/opt/trn_rl_repo")

from contextlib import ExitStack

import ml_dtypes
import numpy as np

import concourse.bacc as bacc
import concourse.bass as bass
import concourse.hw_specs as _hw_specs
import concourse.tile as tile
from concourse import mybir
from concourse.bass_utils import run_bass_kernel_spmd

# Bias the activation-table chooser so Exp resolves to the set that also
# contains Ln ("natural_log_exp_and_others"): the kernel interleaves Exp
# (softmax) with Ln (reciprocal via exp(-ln s)), and per-function minimal
# sets would thrash the ~2.7us ACT table load on every switch.
_orig_get_activation_tables = _hw_specs.get_activation_tables


def _patched_activation_tables(arch):
    exp_fn = mybir.ActivationFunctionType.Exp
    out = {}
    for name, fns in _orig_get_activation_tables(arch).items():
        if name != "natural_log_exp_and_others" and exp_fn in fns:
            fns = [f for f in fns if f != exp_fn]
        out[name] = set(fns)
    return out


bacc.get_activation_tables = _patched_activation_tables

B = 4
S = 2048
D = 1024
H = 16
DH = 64

P = 128
DIN_C = D // P        # 8 bf16 contraction chunks for the projections
K4 = 4                # 4 DoubleRow contraction chunks (256 each)
CC = 4                # channel chunks per core (512 / 128)
NHEAD = 8             # heads per core
QT = S // 512         # query tiles of 512
TT = S // 512         # token tiles of 512
VH = 65               # per-head V block: 64 dims + 1 ones column
VH8 = 72              # fp8 V block padded so the pair-axis stride is 16B-aligned

F32 = mybir.dt.float32
BF16 = mybir.dt.bfloat16
F8 = mybir.dt.float8e4
NP_F8 = ml_dtypes.float8_e4m3
DR = mybir.MatmulPerfMode.DoubleRow

LAST_RESULT = None
_NC_CACHE = None


def build_nc():
    nc = bacc.Bacc()

    # host-prearranged layouts: every slice DMA'd is contiguous per
    # partition (see kernel() for the exact index maps).
    xt8_d = nc.dram_tensor("xt8", [P, TT, K4, 2, 512], F8, kind="ExternalInput")
    xt0_d = nc.dram_tensor("xt0", [P, DIN_C, 512], BF16, kind="ExternalInput")
    wq8_d = nc.dram_tensor("wq8", [CC, P, K4, 2, 128], F8, kind="ExternalInput")
    wk8_d = nc.dram_tensor("wk8", [CC, P, K4, 2, 128], F8, kind="ExternalInput")
    wvt_d = nc.dram_tensor("wvt", [P, DIN_C, 512], BF16, kind="ExternalInput")
    wv8_d = nc.dram_tensor("wv8", [P, K4, 2, 512], F8, kind="ExternalInput")
    wot_d = nc.dram_tensor("wot", [P, CC, D], BF16, kind="ExternalInput")
    wot8_d = nc.dram_tensor("wot8", [P, 2, 2, D], F8, kind="ExternalInput")
    mask_d = nc.dram_tensor("mask", [P, 4, 512], BF16, kind="ExternalInput")
    ones_d = nc.dram_tensor("ones", [P, (S // P) * NHEAD * VH], BF16, kind="ExternalInput")
    ones8_d = nc.dram_tensor("ones8", [P, (S // P) // 2 * 2 * NHEAD * VH8], F8, kind="ExternalInput")
    yt_d = nc.dram_tensor("yt", [D, S], F32, kind="ExternalOutput")

    yt_v = yt_d[:, :]

    EXP = mybir.ActivationFunctionType.Exp

    with tile.TileContext(nc) as tc, ExitStack() as ctx:
        singles = ctx.enter_context(tc.tile_pool(name="singles", bufs=1))
        qk_pool = ctx.enter_context(tc.tile_pool(name="qkp", bufs=2))
        w_pool = ctx.enter_context(tc.tile_pool(name="wp", bufs=2))
        p_pool = ctx.enter_context(tc.tile_pool(name="pp", bufs=3))
        misc = ctx.enter_context(tc.tile_pool(name="misc", bufs=2))
        yt_pool = ctx.enter_context(tc.tile_pool(name="ytp", bufs=2))
        proj_ps = ctx.enter_context(tc.tile_pool(name="proj_ps", bufs=2, space="PSUM"))
        sps_ps = ctx.enter_context(tc.tile_pool(name="sps_ps", bufs=2, space="PSUM"))
        ops_ps = ctx.enter_context(tc.tile_pool(name="ops_ps", bufs=1, space="PSUM"))
        dram_pool = ctx.enter_context(tc.tile_pool(name="drp", bufs=2, space="DRAM"))

        xt8_sb = singles.tile([P, TT, K4, 2, 512], F8)
        xt0_sb = singles.tile([P, DIN_C, 512], BF16)
        v_sb = singles.tile([P, S // P, NHEAD, VH], BF16)
        v8_sb = singles.tile([P, S // P // 2, 2, NHEAD, VH8], F8)
        ot_sb = singles.tile([P, CC, 512], BF16)
        ot8_sb = singles.tile([P, 2, 2, S], F8)
        wot_sb = singles.tile([P, CC, D], BF16)
        wot8_sb = singles.tile([P, 2, 2, D], F8)
        mask_sb = singles.tile([P, 4, 512], BF16)
        wv_sb = singles.tile([P, DIN_C, 512], BF16)
        wv8_sb = singles.tile([P, K4, 2, 512], F8)

        qk_tiles = {}

        def make_qk(cc):
            wq_sb = w_pool.tile([P, K4, 2, 128], F8, tag="wq", name=f"wq_{cc}")
            wk_sb = w_pool.tile([P, K4, 2, 128], F8, tag="wk", name=f"wk_{cc}")
            nc.sync.dma_start(out=wq_sb, in_=wq8_d[cc])
            nc.sync.dma_start(out=wk_sb, in_=wk8_d[cc])
            qt_sb = qk_pool.tile([P, S], BF16, tag="qt", name=f"qtsb_{cc}")
            kt_sb = qk_pool.tile([P, S], BF16, tag="kt", name=f"ktsb_{cc}")
            qk_tiles[cc] = (wq_sb, wk_sb, qt_sb, kt_sb)

        def proj_chunks(cc, tt):
            """Return compute thunks (one PSUM group each) to interleave
            between attention half-groups.  X is already in SBUF."""
            wq_sb, wk_sb, qt_sb, kt_sb = qk_tiles[cc]
            thunks = []
            if cc == 0:
                for sub in range(4):
                    def vthunk(sub=sub, tt=tt):
                        vps = proj_ps.tile([P, 512], F32, tag="pp",
                                           name=f"vps_{tt}_{sub}")
                        if tt == 0:
                            for kc in range(DIN_C):
                                nc.tensor.matmul(
                                    vps,
                                    xt0_sb[:, kc, sub * 128:(sub + 1) * 128],
                                    wv_sb[:, kc, :],
                                    start=(kc == 0),
                                    stop=(kc == DIN_C - 1),
                                )
                        else:
                            for k4 in range(K4):
                                nc.tensor.matmul(
                                    vps,
                                    xt8_sb[:, tt, k4, :, sub * 128:(sub + 1) * 128],
                                    wv8_sb[:, k4],
                                    start=(k4 == 0),
                                    stop=(k4 == K4 - 1),
                                    perf_mode=DR,
                                )
                        tch = tt * 4 + sub
                        nc.vector.tensor_copy(v_sb[:, tch, :, 0:64], vps)
                        nc.vector.tensor_copy(
                            v8_sb[:, tch // 2, tch % 2, :, 0:64], vps)
                    thunks.append(vthunk)

            def qthunk(tt=tt, cc=cc, wq_sb=wq_sb, qt_sb=qt_sb):
                qps = proj_ps.tile([P, 512], F32, tag="pp", name=f"qps_{cc}_{tt}")
                for k4 in range(K4):
                    nc.tensor.matmul(
                        qps, wq_sb[:, k4], xt8_sb[:, tt, k4],
                        start=(k4 == 0), stop=(k4 == K4 - 1),
                        perf_mode=DR,
                    )
                nc.vector.tensor_copy(qt_sb[:, tt * 512:(tt + 1) * 512], qps)

            def kthunk(tt=tt, cc=cc, wk_sb=wk_sb, kt_sb=kt_sb):
                kps = proj_ps.tile([P, 512], F32, tag="pp", name=f"kps_{cc}_{tt}")
                for k4 in range(K4):
                    nc.tensor.matmul(
                        kps, wk_sb[:, k4], xt8_sb[:, tt, k4],
                        start=(k4 == 0), stop=(k4 == K4 - 1),
                        perf_mode=DR,
                    )
                nc.vector.tensor_copy(kt_sb[:, tt * 512:(tt + 1) * 512], kps)

            thunks += [qthunk, kthunk]
            return thunks

        def oproj_chunk(tt_o, oc):
            def th():
                ops_o = proj_ps.tile([P, 512], F32, tag="pp",
                                     name=f"ops_o_{tt_o}_{oc}")
                if tt_o == 0:
                    for c2 in range(CC):
                        nc.tensor.matmul(
                            ops_o,
                            wot_sb[:, c2, oc * 128:(oc + 1) * 128],
                            ot_sb[:, c2, :],
                            start=(c2 == 0),
                            stop=(c2 == CC - 1),
                        )
                else:
                    for c2 in range(2):
                        nc.tensor.matmul(
                            ops_o,
                            wot8_sb[:, c2, :, oc * 128:(oc + 1) * 128],
                            ot8_sb[:, c2, :, tt_o * 512:(tt_o + 1) * 512],
                            start=(c2 == 0),
                            stop=(c2 == 1),
                            perf_mode=DR,
                        )
                y_t = yt_pool.tile([P, 512], F32, tag="yt",
                                   name=f"yt_{tt_o}_{oc}")
                nc.vector.tensor_copy(y_t, ops_o)
                nc.gpsimd.dma_start(
                    out=yt_v[oc * 128:(oc + 1) * 128,
                             tt_o * 512:(tt_o + 1) * 512],
                    in_=y_t,
                )
            return th

        # ---- prologue: the first projection's operands (xt0 + W_V) are
        # split across queues so nothing serializes behind a megabyte;
        # Q/K weights + X tile 0 go ahead of the bulk X transfer. ----
        nc.sync.dma_start(out=xt0_sb[:, 0:4, :], in_=xt0_d[:, 0:4, :])
        nc.gpsimd.dma_start(out=wv_sb[:, 0:4, :], in_=wvt_d[:, 0:4, :])
        make_qk(0)
        nc.sync.dma_start(out=xt0_sb[:, 4:8, :], in_=xt0_d[:, 4:8, :])
        nc.gpsimd.dma_start(out=wv_sb[:, 4:8, :], in_=wvt_d[:, 4:8, :])
        nc.sync.dma_start(out=xt8_sb[:, 0], in_=xt8_d[:, 0])
        nc.gpsimd.dma_start(out=wv8_sb, in_=wv8_d[:, :, :, :])
        for tt in range(1, TT):
            nc.sync.dma_start(out=xt8_sb[:, tt], in_=xt8_d[:, tt])
        nc.gpsimd.dma_start(out=mask_sb, in_=mask_d[:, :, :])
        # fill v_sb with 1.0 (per token-chunk group so the V copies unblock
        # progressively); the V projection overwrites the data columns,
        # leaving col 64 of each head block as the ones column.
        for q4 in range(4):
            nc.gpsimd.dma_start(
                out=v_sb[:, q4 * 4:(q4 + 1) * 4, :, :],
                in_=ones_d[:, q4 * 4 * NHEAD * VH:(q4 + 1) * 4 * NHEAD * VH],
            )
            nc.gpsimd.dma_start(
                out=v8_sb[:, q4 * 2:(q4 + 1) * 2, :, :, :],
                in_=ones8_d[:, q4 * 2 * 2 * NHEAD * VH8:
                            (q4 + 1) * 2 * 2 * NHEAD * VH8],
            )
        nc.gpsimd.dma_start(out=wot_sb, in_=wot_d[:, :, :])
        nc.gpsimd.dma_start(out=wot8_sb, in_=wot8_d[:, :, :, :])
        # pre-zero the diagonal-P buffers: the trimmed exp skips the fully
        # masked query range, and the mask-multiply must see finite values
        # (0 * garbage) there on the first three uses of each buffer.
        for z in range(3):
            zt = p_pool.tile([P, 2, 512], BF16, tag="p", name=f"pz_{z}")
            nc.gpsimd.memset(zt, 0.0)
        for th in proj_chunks(0, 0):
            th()

        PIPE = 2  # PV runs this many half-groups behind the scores

        for cc in range(CC):
            _, _, qt_sb, kt_sb = qk_tiles[cc]
            # The last channel chunk walks its query tiles as [0, 3, 2, 1]:
            # each finished tile's output projection then overlaps the next
            # tile's attention, leaving only tile 1's oproj for the tail.
            qt_order = list(range(TT)) if cc < CC - 1 else [0, 3, 2, 1]
            for step, qt in enumerate(qt_order):
                fillers = []
                if cc < CC - 1:
                    if qt < TT - 1:
                        fillers += proj_chunks(cc, qt + 1)
                    else:
                        make_qk(cc + 1)
                        fillers += proj_chunks(cc + 1, 0)
                else:
                    if step == 0:
                        for tt in range(1, TT):
                            fillers += proj_chunks(CC - 1, tt)
                    else:
                        prev_tile = [None, 0, 3, 2][step]
                        fillers += [oproj_chunk(prev_tile, oc)
                                    for oc in range(D // P)]
                fillers = list(fillers)
                fi = 0

                nk = 4 * qt + 4
                qsl = slice(qt * 512, (qt + 1) * 512)
                ops_both = ops_ps.tile([P, 2, 512], F32, tag="ops",
                                       name=f"ops_{cc}_{qt}")
                pvq = []

                def emit_pv(entry):
                    kind, a, t = entry
                    if kind == "single":
                        for h2 in range(2):
                            nc.tensor.matmul(
                                ops_both[0:VH, h2, :],
                                v_sb[:, a, 2 * cc + h2, 0:VH],
                                t[:, h2, :],
                                start=(a == 0),
                                stop=(a == nk - 1),
                                skip_group_check=True,
                            )
                    else:  # fp8 DoubleRow pair: two key chunks per matmul
                        for h2 in range(2):
                            nc.tensor.matmul(
                                ops_both[0:VH, h2, :],
                                v8_sb[:, a, :, 2 * cc + h2, 0:VH],
                                t[:, :, h2, :],
                                start=(a == 0),
                                stop=False,
                                perf_mode=DR,
                                skip_group_check=True,
                            )

                pp_cur = None
                for kc in range(nk):
                    sps = sps_ps.tile([P, 2, 512], F32, tag="sps",
                                      name=f"sps_{cc}_{qt}_{kc}")
                    for h2 in range(2):
                        b0 = h2 * 64
                        nc.tensor.matmul(
                            sps[:, h2, :],
                            kt_sb[b0:b0 + 64, kc * 128:(kc + 1) * 128],
                            qt_sb[b0:b0 + 64, qsl],
                            start=True,
                            stop=True,
                        )
                    if kc < 4 * qt:
                        # sub-diagonal: exp into one slot of an fp8 pair tile
                        if kc % 2 == 0:
                            pp_cur = p_pool.tile([P, 2, 2, 512], F8, tag="p8",
                                                 name=f"pp_{cc}_{qt}_{kc}")
                        nc.scalar.activation(pp_cur[:, kc % 2], sps,
                                             EXP, scale=0.125)
                        if kc % 2 == 1:
                            pvq.append(("pair", kc // 2, pp_cur))
                    else:
                        # diagonal chunk: bf16 exp over the causally valid
                        # query range only (queries below r*128 are fully
                        # masked; the mask-multiply zeroes the stale values)
                        r = kc - 4 * qt
                        p_t = p_pool.tile([P, 2, 512], BF16, tag="p",
                                          name=f"p_{cc}_{qt}_{kc}")
                        nc.scalar.activation(p_t[:, :, r * 128:512],
                                             sps[:, :, r * 128:512],
                                             EXP, scale=0.125)
                        for h2 in range(2):
                            nc.gpsimd.tensor_mul(
                                p_t[:, h2, :],
                                p_t[:, h2, :],
                                mask_sb[:, r, :],
                            )
                        pvq.append(("single", kc, p_t))
                    if len(pvq) > PIPE:
                        emit_pv(pvq.pop(0))
                    # keep PE fed while ScalarE works through the exps
                    if kc % 2 == 1 and fi < len(fillers):
                        fillers[fi]()
                        fi += 1
                for item in pvq:
                    emit_pv(item)

                # Move U out of PSUM (frees the ops banks), compute
                # 1/s = exp(-ln s) for both heads in one ScalarE chain,
                # broadcast across 64 partitions via a DRAM-bounce DMA,
                # then scale U on VectorE.
                u_sb = misc.tile([64, 2, 512], F32, tag="u",
                                 name=f"u_{cc}_{qt}")
                nc.vector.tensor_copy(u_sb, ops_both[0:64, :, :])
                rrow = misc.tile([P, 2, 512], F32, tag="rrow",
                                 name=f"rrow_{cc}_{qt}")
                nc.scalar.activation(rrow[64:65, :, :], ops_both[64:65, :, :],
                                     mybir.ActivationFunctionType.Ln)
                rexp = misc.tile([P, 2, 512], F32, tag="rexp",
                                 name=f"rexp_{cc}_{qt}")
                nc.scalar.activation(rexp[64:65, :, :], rrow[64:65, :, :],
                                     EXP, scale=-1.0)
                rdram = dram_pool.tile([1, 1024], F32, tag="rd",
                                       name=f"rd_{cc}_{qt}")
                nc.gpsimd.dma_start(out=rdram, in_=rexp[64:65, :, :])
                rec = misc.tile([64, 1024], F32, tag="rec",
                                name=f"rec_{cc}_{qt}")
                rsrc = rdram[0:1, :]
                nc.gpsimd.dma_start(
                    out=rec,
                    in_=bass.AP(tensor=rsrc.tensor, offset=rsrc.offset,
                                ap=[[0, 64], [1, 1024]]),
                )
                for h2 in range(2):
                    if qt == 0:
                        o_dst = ot_sb[h2 * 64:h2 * 64 + 64, cc, :]
                    else:
                        o_dst = ot8_sb[h2 * 64:h2 * 64 + 64, cc // 2,
                                       cc % 2, qsl]
                    nc.vector.tensor_mul(o_dst, u_sb[:, h2, :],
                                         rec[:, h2 * 512:(h2 + 1) * 512])
                # leftover independent work lands here
                while fi < len(fillers):
                    fillers[fi]()
                    fi += 1

        # tail: the one remaining output-projection tile
        for oc in range(D // P):
            oproj_chunk(1, oc)()

    nc.finalize()
    return nc


def _make_mask():
    keys = np.arange(4)[None, :, None] * 128 + np.arange(128)[:, None, None]
    qs = np.arange(512)[None, None, :]
    return (keys <= qs).astype(np.float32)


def _to_f8(a):
    return np.clip(a, -240.0, 240.0).astype(NP_F8)


def kernel(X, W_Q, W_K, W_V, W_O):
    global LAST_RESULT, _NC_CACHE
    X = np.asarray(X, dtype=np.float32)
    W_Q = np.asarray(W_Q, dtype=np.float32)
    W_K = np.asarray(W_K, dtype=np.float32)
    W_V = np.asarray(W_V, dtype=np.float32)
    W_O = np.asarray(W_O, dtype=np.float32)

    mask = _make_mask().astype(ml_dtypes.bfloat16)
    in_maps = []
    for c in range(8):
        b, g = c // 2, c % 2
        sl = slice(g * 512, (g + 1) * 512)
        XT = X[b].T  # [d, t]
        # DoubleRow fold: d = k4*256 + i*128 + p -> xt8[p, tt, k4, i, ti]
        xt8 = np.ascontiguousarray(
            XT.reshape(K4, 2, P, TT, 512).transpose(2, 3, 0, 1, 4)
        )
        xt8 = _to_f8(xt8)
        # bf16 X, token tile 0 only (for the bf16 V projection):
        # d = kc*128 + p -> xt0[p, kc, ti]
        xt0 = np.ascontiguousarray(
            XT[:, 0:512].reshape(DIN_C, P, 512).transpose(1, 0, 2)
        ).astype(ml_dtypes.bfloat16)
        # W[sl,:].T is [d_in, c_out]; DoubleRow fold on d_in, c_out split
        # by channel chunk: -> w8[cc, p, k4, i, ci]
        wq8 = _to_f8(np.ascontiguousarray(
            W_Q[sl, :].T.reshape(K4, 2, P, CC, 128).transpose(3, 2, 0, 1, 4)
        ))
        wk8 = _to_f8(np.ascontiguousarray(
            W_K[sl, :].T.reshape(K4, 2, P, CC, 128).transpose(3, 2, 0, 1, 4)
        ))
        # W_V[sl,:].T bf16 -> [p, kc, c] and fp8 DoubleRow -> [p, k4, i, c]
        WVT = W_V[sl, :].T
        wv = np.ascontiguousarray(
            WVT.reshape(DIN_C, P, 512).transpose(1, 0, 2)
        ).astype(ml_dtypes.bfloat16)
        wv8 = _to_f8(np.ascontiguousarray(
            WVT.reshape(K4, 2, P, 512).transpose(2, 0, 1, 3)
        ))
        # W_O[:, sl].T is [c_in(512), o(1024)]; c_in = cc*128 + p -> [p, cc, o]
        WOT = W_O[:, sl].T
        wo = np.ascontiguousarray(
            WOT.reshape(CC, P, D).transpose(1, 0, 2)
        ).astype(ml_dtypes.bfloat16)
        # DoubleRow fold for the fp8 O-projection: c_in = c2*256 + i*128 + p
        wo8 = _to_f8(np.ascontiguousarray(
            WOT.reshape(2, 2, P, D).transpose(2, 0, 1, 3)
        ))
        in_maps.append({
            "ones": np.ones((128, 16 * 8 * 65), dtype=ml_dtypes.bfloat16),
            "ones8": np.ones((128, 8 * 2 * 8 * VH8), dtype=NP_F8),
            "wot8": wo8,
            "xt8": xt8,
            "xt0": xt0,
            "wq8": wq8,
            "wk8": wk8,
            "wvt": wv,
            "wv8": wv8,
            "wot": wo,
            "mask": mask,
        })

    if _NC_CACHE is None:
        _NC_CACHE = build_nc()
    res = run_bass_kernel_spmd(_NC_CACHE, in_maps, core_ids=list(range(8)))
    LAST_RESULT = res

    out = np.empty((B, S, D), dtype=np.float32)
    for b in range(B):
        yt = res.results[2 * b]["yt"] + res.results[2 * b + 1]["yt"]
        out[b] = yt.T
    return out


# revision 25
# speedup vs baseline: 1.2049x; 1.2049x over previous
"""Causal multi-head self-attention on 8 Trainium2 NeuronCores.

Problem: X[4, 2048, 1024] fp32, W_Q/W_K/W_V/W_O [1024, 1024] fp32,
16 heads x 64 dims, causal softmax attention + output projection.

Sharding: core c handles batch b = c//2 and head-group g = c%2
(heads g*8..g*8+8, i.e. 512 of the 1024 channels).  Each core computes
its 8 heads' Q/K/V projections, causal attention, and a partial output
projection against W_O[:, g*512:(g+1)*512]; the host sums the two
partial outputs per batch (the "all-reduce after W_O" step).

Device kernel layout notes (v3 = v2 schedule + fp8 DoubleRow projections):
 - Q/K projections (all token tiles) and the V projection (token tiles
   1-3) run as fp8e4m3 DoubleRow matmuls: contraction 256 per matmul
   (128 partitions x 2 packed), halving the matmul count.  Inputs are
   host-quantized; accumulation stays fp32 in PSUM, and Q/K/V are
   written back as bf16, so only the *inputs* of these GEMMs are fp8.
   V for tokens 0-511 stays bf16 (early queries put ~all softmax weight
   on the first keys, so those V rows need bf16 accuracy; later keys
   are averaged over >=512 positions and fp8 noise washes out).
 - Attention (scores, exp, PV) is bf16 throughout: with a 64/128-wide
   contraction DoubleRow cannot reduce the streamed row count, so fp8
   would win nothing there.
 - The two per-chunk score matmuls use disjoint contraction partitions
   (heads at 0-63 / 64-127) and run concurrently on the PE array via
   row tiling.
 - Attention runs in "half-groups" of ONE 128-key chunk covering both
   heads, double-buffered score PSUM, PV software-pipelined two
   half-groups behind, projection/output-projection thunks interleaved
   between half-groups (see v2 notes in git history).
   PSUM budget: 4 (scores) + 2 (ops) + 2 (proj) = 8 banks.
 - All host-side tensors are pre-arranged so every DMA is >=1KB
   contiguous per partition; prologue DMAs are spread across the
   sync/vector/scalar/gpsimd queues so the first projection's operands
   (xt tile 0 + W_V) don't serialize behind each other.
 - Softmax skips the max-subtraction (scores are bounded ~|1.9| after
   the 1/8 scale); causal masking multiplies the diagonal score chunks
   by a 0/1 mask after exp; V carries a ones column so PSUM rows 64
   hold the softmax row-sums (normalization = reciprocal + multiply).
"""

import sys

if "/opt/trn_rl_repo" not in sys.path:
    sys.path.insert(0, "/opt/trn_rl_repo")

from contextlib import ExitStack

import ml_dtypes
import numpy as np

import concourse.bacc as bacc
import concourse.bass as bass
import concourse.hw_specs as _hw_specs
import concourse.tile as tile
from concourse import mybir
from concourse.bass_utils import run_bass_kernel_spmd

# Bias the activation-table chooser so Exp resolves to the set that also
# contains Ln ("natural_log_exp_and_others"): the kernel interleaves Exp
# (softmax) with Ln (reciprocal via exp(-ln s)), and per-function minimal
# sets would thrash the ~2.7us ACT table load on every switch.
_orig_get_activation_tables = _hw_specs.get_activation_tables


def _patched_activation_tables(arch):
    exp_fn = mybir.ActivationFunctionType.Exp
    out = {}
    for name, fns in _orig_get_activation_tables(arch).items():
        if name != "natural_log_exp_and_others" and exp_fn in fns:
            fns = [f for f in fns if f != exp_fn]
        out[name] = set(fns)
    return out


bacc.get_activation_tables = _patched_activation_tables

B = 4
S = 2048
D = 1024
H = 16
DH = 64

P = 128
DIN_C = D // P        # 8 bf16 contraction chunks for the projections
K4 = 4                # 4 DoubleRow contraction chunks (256 each)
CC = 4                # channel chunks per core (512 / 128)
NHEAD = 8             # heads per core
QT = S // 512         # query tiles of 512
TT = S // 512         # token tiles of 512
VH = 65               # per-head V block: 64 dims + 1 ones column
VH8 = 72              # fp8 V block padded so the pair-axis stride is 16B-aligned

F32 = mybir.dt.float32
BF16 = mybir.dt.bfloat16
F8 = mybir.dt.float8e4
NP_F8 = ml_dtypes.float8_e4m3
DR = mybir.MatmulPerfMode.DoubleRow

LAST_RESULT = None
_NC_CACHE = None


def build_nc():
    nc = bacc.Bacc()

    # host-prearranged layouts: every slice DMA'd is contiguous per
    # partition (see kernel() for the exact index maps).
    xt8_d = nc.dram_tensor("xt8", [P, TT, K4, 2, 512], F8, kind="ExternalInput")
    xt0_d = nc.dram_tensor("xt0", [P, DIN_C, 512], BF16, kind="ExternalInput")
    wq8_d = nc.dram_tensor("wq8", [CC, P, K4, 2, 128], F8, kind="ExternalInput")
    wk8_d = nc.dram_tensor("wk8", [CC, P, K4, 2, 128], F8, kind="ExternalInput")
    wvt_d = nc.dram_tensor("wvt", [P, DIN_C, 512], BF16, kind="ExternalInput")
    wv8_d = nc.dram_tensor("wv8", [P, K4, 2, 512], F8, kind="ExternalInput")
    wot_d = nc.dram_tensor("wot", [P, CC, D], BF16, kind="ExternalInput")
    wot8_d = nc.dram_tensor("wot8", [P, 2, 2, D], F8, kind="ExternalInput")
    mask_d = nc.dram_tensor("mask", [P, 4, 512], BF16, kind="ExternalInput")
    ones_d = nc.dram_tensor("ones", [P, (S // P) * NHEAD * VH], BF16, kind="ExternalInput")
    ones8_d = nc.dram_tensor("ones8", [P, (S // P) // 2 * 2 * NHEAD * VH8], F8, kind="ExternalInput")
    yt_d = nc.dram_tensor("yt", [D, S], F32, kind="ExternalOutput")

    yt_v = yt_d[:, :]

    EXP = mybir.ActivationFunctionType.Exp

    with tile.TileContext(nc) as tc, ExitStack() as ctx:
        singles = ctx.enter_context(tc.tile_pool(name="singles", bufs=1))
        qk_pool = ctx.enter_context(tc.tile_pool(name="qkp", bufs=2))
        w_pool = ctx.enter_context(tc.tile_pool(name="wp", bufs=2))
        p_pool = ctx.enter_context(tc.tile_pool(name="pp", bufs=3))
        misc = ctx.enter_context(tc.tile_pool(name="misc", bufs=2))
        yt_pool = ctx.enter_context(tc.tile_pool(name="ytp", bufs=2))
        proj_ps = ctx.enter_context(tc.tile_pool(name="proj_ps", bufs=2, space="PSUM"))
        sps_ps = ctx.enter_context(tc.tile_pool(name="sps_ps", bufs=2, space="PSUM"))
        ops_ps = ctx.enter_context(tc.tile_pool(name="ops_ps", bufs=1, space="PSUM"))
        dram_pool = ctx.enter_context(tc.tile_pool(name="drp", bufs=2, space="DRAM"))

        xt8_sb = singles.tile([P, TT, K4, 2, 512], F8)
        xt0_sb = singles.tile([P, DIN_C, 512], BF16)
        v_sb = singles.tile([P, S // P, NHEAD, VH], BF16)
        v8_sb = singles.tile([P, S // P // 2, 2, NHEAD, VH8], F8)
        ot_sb = singles.tile([P, CC, 512], BF16)
        ot8_sb = singles.tile([P, 2, 2, S], F8)
        wot_sb = singles.tile([P, CC, D], BF16)
        wot8_sb = singles.tile([P, 2, 2, D], F8)
        mask_sb = singles.tile([P, 4, 512], BF16)
        wv_sb = singles.tile([P, DIN_C, 512], BF16)
        wv8_sb = singles.tile([P, K4, 2, 512], F8)

        qk_tiles = {}

        def make_qk(cc):
            wq_sb = w_pool.tile([P, K4, 2, 128], F8, tag="wq", name=f"wq_{cc}")
            wk_sb = w_pool.tile([P, K4, 2, 128], F8, tag="wk", name=f"wk_{cc}")
            nc.sync.dma_start(out=wq_sb, in_=wq8_d[cc])
            nc.sync.dma_start(out=wk_sb, in_=wk8_d[cc])
            qt_sb = qk_pool.tile([P, S], BF16, tag="qt", name=f"qtsb_{cc}")
            kt_sb = qk_pool.tile([P, S], BF16, tag="kt", name=f"ktsb_{cc}")
            qk_tiles[cc] = (wq_sb, wk_sb, qt_sb, kt_sb)

        def proj_chunks(cc, tt):
            """Return compute thunks (one PSUM group each) to interleave
            between attention half-groups.  X is already in SBUF."""
            wq_sb, wk_sb, qt_sb, kt_sb = qk_tiles[cc]
            thunks = []
            if cc == 0:
                for sub in range(4):
                    def vthunk(sub=sub, tt=tt):
                        vps = proj_ps.tile([P, 512], F32, tag="pp",
                                           name=f"vps_{tt}_{sub}")
                        if tt == 0:
                            for kc in range(DIN_C):
                                nc.tensor.matmul(
                                    vps,
                                    xt0_sb[:, kc, sub * 128:(sub + 1) * 128],
                                    wv_sb[:, kc, :],
                                    start=(kc == 0),
                                    stop=(kc == DIN_C - 1),
                                )
                        else:
                            for k4 in range(K4):
                                nc.tensor.matmul(
                                    vps,
                                    xt8_sb[:, tt, k4, :, sub * 128:(sub + 1) * 128],
                                    wv8_sb[:, k4],
                                    start=(k4 == 0),
                                    stop=(k4 == K4 - 1),
                                    perf_mode=DR,
                                )
                        tch = tt * 4 + sub
                        nc.vector.tensor_copy(v_sb[:, tch, :, 0:64], vps)
                        nc.vector.tensor_copy(
                            v8_sb[:, tch // 2, tch % 2, :, 0:64], vps)
                    thunks.append(vthunk)

            def qthunk(tt=tt, cc=cc, wq_sb=wq_sb, qt_sb=qt_sb):
                qps = proj_ps.tile([P, 512], F32, tag="pp", name=f"qps_{cc}_{tt}")
                for k4 in range(K4):
                    nc.tensor.matmul(
                        qps, wq_sb[:, k4], xt8_sb[:, tt, k4],
                        start=(k4 == 0), stop=(k4 == K4 - 1),
                        perf_mode=DR,
                    )
                nc.vector.tensor_copy(qt_sb[:, tt * 512:(tt + 1) * 512], qps)

            def kthunk(tt=tt, cc=cc, wk_sb=wk_sb, kt_sb=kt_sb):
                kps = proj_ps.tile([P, 512], F32, tag="pp", name=f"kps_{cc}_{tt}")
                for k4 in range(K4):
                    nc.tensor.matmul(
                        kps, wk_sb[:, k4], xt8_sb[:, tt, k4],
                        start=(k4 == 0), stop=(k4 == K4 - 1),
                        perf_mode=DR,
                    )
                nc.vector.tensor_copy(kt_sb[:, tt * 512:(tt + 1) * 512], kps)

            thunks += [qthunk, kthunk]
            return thunks

        def oproj_chunk(tt_o, oc):
            def th():
                ops_o = proj_ps.tile([P, 512], F32, tag="pp",
                                     name=f"ops_o_{tt_o}_{oc}")
                if tt_o == 0:
                    for c2 in range(CC):
                        nc.tensor.matmul(
                            ops_o,
                            wot_sb[:, c2, oc * 128:(oc + 1) * 128],
                            ot_sb[:, c2, :],
                            start=(c2 == 0),
                            stop=(c2 == CC - 1),
                        )
                else:
                    for c2 in range(2):
                        nc.tensor.matmul(
                            ops_o,
                            wot8_sb[:, c2, :, oc * 128:(oc + 1) * 128],
                            ot8_sb[:, c2, :, tt_o * 512:(tt_o + 1) * 512],
                            start=(c2 == 0),
                            stop=(c2 == 1),
                            perf_mode=DR,
                        )
                y_t = yt_pool.tile([P, 512], F32, tag="yt",
                                   name=f"yt_{tt_o}_{oc}")
                nc.vector.tensor_copy(y_t, ops_o)
                nc.gpsimd.dma_start(
                    out=yt_v[oc * 128:(oc + 1) * 128,
                             tt_o * 512:(tt_o + 1) * 512],
                    in_=y_t,
                )
            return th

        # ---- prologue: the first projection's operands (xt0 + W_V) are
        # split across queues so nothing serializes behind a megabyte;
        # Q/K weights + X tile 0 go ahead of the bulk X transfer. ----
        nc.sync.dma_start(out=xt0_sb[:, 0:4, :], in_=xt0_d[:, 0:4, :])
        nc.gpsimd.dma_start(out=wv_sb[:, 0:4, :], in_=wvt_d[:, 0:4, :])
        make_qk(0)
        nc.sync.dma_start(out=xt0_sb[:, 4:8, :], in_=xt0_d[:, 4:8, :])
        nc.gpsimd.dma_start(out=wv_sb[:, 4:8, :], in_=wvt_d[:, 4:8, :])
        nc.sync.dma_start(out=xt8_sb[:, 0], in_=xt8_d[:, 0])
        nc.gpsimd.dma_start(out=wv8_sb, in_=wv8_d[:, :, :, :])
        for tt in range(1, TT):
            nc.sync.dma_start(out=xt8_sb[:, tt], in_=xt8_d[:, tt])
        nc.gpsimd.dma_start(out=mask_sb, in_=mask_d[:, :, :])
        # fill v_sb with 1.0 (per token-chunk group so the V copies unblock
        # progressively); the V projection overwrites the data columns,
        # leaving col 64 of each head block as the ones column.
        for q4 in range(4):
            nc.gpsimd.dma_start(
                out=v_sb[:, q4 * 4:(q4 + 1) * 4, :, :],
                in_=ones_d[:, q4 * 4 * NHEAD * VH:(q4 + 1) * 4 * NHEAD * VH],
            )
            nc.gpsimd.dma_start(
                out=v8_sb[:, q4 * 2:(q4 + 1) * 2, :, :, :],
                in_=ones8_d[:, q4 * 2 * 2 * NHEAD * VH8:
                            (q4 + 1) * 2 * 2 * NHEAD * VH8],
            )
        nc.gpsimd.dma_start(out=wot_sb, in_=wot_d[:, :, :])
        nc.gpsimd.dma_start(out=wot8_sb, in_=wot8_d[:, :, :, :])
        # pre-zero the diagonal-P buffers: the trimmed exp skips the fully
        # masked query range, and the mask-multiply must see finite values
        # (0 * garbage) there on the first three uses of each buffer.
        for z in range(3):
            zt = p_pool.tile([P, 2, 512], BF16, tag="p", name=f"pz_{z}")
            nc.gpsimd.memset(zt, 0.0)
        for th in proj_chunks(0, 0):
            th()

        PIPE = 2  # PV runs this many half-groups behind the scores

        for cc in range(CC):
            _, _, qt_sb, kt_sb = qk_tiles[cc]
            # The last channel chunk walks its query tiles as [0, 3, 2, 1]:
            # each finished tile's output projection then overlaps the next
            # tile's attention, leaving only tile 1's oproj for the tail.
            qt_order = list(range(TT)) if cc < CC - 1 else [0, 3, 2, 1]
            for step, qt in enumerate(qt_order):
                fillers = []
                if cc < CC - 1:
                    if qt < TT - 1:
                        fillers += proj_chunks(cc, qt + 1)
                    else:
                        make_qk(cc + 1)
                        fillers += proj_chunks(cc + 1, 0)
                else:
                    if step == 0:
                        for tt in range(1, TT):
                            fillers += proj_chunks(CC - 1, tt)
                    else:
                        prev_tile = [None, 0, 3, 2][step]
                        fillers += [oproj_chunk(prev_tile, oc)
                                    for oc in range(D // P)]
                fillers = list(fillers)
                fi = 0

                nk = 4 * qt + 4
                qsl = slice(qt * 512, (qt + 1) * 512)
                ops_both = ops_ps.tile([P, 2, 512], F32, tag="ops",
                                       name=f"ops_{cc}_{qt}")
                pvq = []

                def emit_pv(entry):
                    kind, a, t = entry
                    if kind == "single":
                        for h2 in range(2):
                            nc.tensor.matmul(
                                ops_both[0:VH, h2, :],
                                v_sb[:, a, 2 * cc + h2, 0:VH],
                                t[:, h2, :],
                                start=(a == 0),
                                stop=(a == nk - 1),
                                skip_group_check=True,
                            )
                    else:  # fp8 DoubleRow pair: two key chunks per matmul
                        for h2 in range(2):
                            nc.tensor.matmul(
                                ops_both[0:VH, h2, :],
                                v8_sb[:, a, :, 2 * cc + h2, 0:VH],
                                t[:, :, h2, :],
                                start=(a == 0),
                                stop=False,
                                perf_mode=DR,
                                skip_group_check=True,
                            )

                pp_cur = None
                for kc in range(nk):
                    sps = sps_ps.tile([P, 2, 512], F32, tag="sps",
                                      name=f"sps_{cc}_{qt}_{kc}")
                    for h2 in range(2):
                        b0 = h2 * 64
                        nc.tensor.matmul(
                            sps[:, h2, :],
                            kt_sb[b0:b0 + 64, kc * 128:(kc + 1) * 128],
                            qt_sb[b0:b0 + 64, qsl],
                            start=True,
                            stop=True,
                        )
                    if kc < 4 * qt:
                        # sub-diagonal: exp into one slot of an fp8 pair tile
                        if kc % 2 == 0:
                            pp_cur = p_pool.tile([P, 2, 2, 512], F8, tag="p8",
                                                 name=f"pp_{cc}_{qt}_{kc}")
                        nc.scalar.activation(pp_cur[:, kc % 2], sps,
                                             EXP, scale=0.125)
                        if kc % 2 == 1:
                            pvq.append(("pair", kc // 2, pp_cur))
                    else:
                        # diagonal chunk: bf16 exp over the causally valid
                        # query range only (queries below r*128 are fully
                        # masked; the mask-multiply zeroes the stale values)
                        r = kc - 4 * qt
                        p_t = p_pool.tile([P, 2, 512], BF16, tag="p",
                                          name=f"p_{cc}_{qt}_{kc}")
                        nc.scalar.activation(p_t[:, :, r * 128:512],
                                             sps[:, :, r * 128:512],
                                             EXP, scale=0.125)
                        for h2 in range(2):
                            nc.vector.tensor_mul(
                                p_t[:, h2, :],
                                p_t[:, h2, :],
                                mask_sb[:, r, :],
                            )
                        pvq.append(("single", kc, p_t))
                    if len(pvq) > PIPE:
                        emit_pv(pvq.pop(0))
                    # keep PE fed while ScalarE works through the exps
                    if kc % 2 == 1 and fi < len(fillers):
                        fillers[fi]()
                        fi += 1
                for item in pvq:
                    emit_pv(item)

                # Move U out of PSUM (frees the ops banks), compute
                # 1/s = exp(-ln s) for both heads in one ScalarE chain,
                # broadcast across 64 partitions via a DRAM-bounce DMA,
                # then scale U on VectorE.
                u_sb = misc.tile([64, 2, 512], F32, tag="u",
                                 name=f"u_{cc}_{qt}")
                nc.vector.tensor_copy(u_sb, ops_both[0:64, :, :])
                rrow = misc.tile([P, 2, 512], F32, tag="rrow",
                                 name=f"rrow_{cc}_{qt}")
                nc.scalar.activation(rrow[64:65, :, :], ops_both[64:65, :, :],
                                     mybir.ActivationFunctionType.Ln)
                rexp = misc.tile([P, 2, 512], F32, tag="rexp",
                                 name=f"rexp_{cc}_{qt}")
                nc.scalar.activation(rexp[64:65, :, :], rrow[64:65, :, :],
                                     EXP, scale=-1.0)
                rdram = dram_pool.tile([1, 1024], F32, tag="rd",
                                       name=f"rd_{cc}_{qt}")
                nc.gpsimd.dma_start(out=rdram, in_=rexp[64:65, :, :])
                rec = misc.tile([64, 1024], F32, tag="rec",
                                name=f"rec_{cc}_{qt}")
                rsrc = rdram[0:1, :]
                nc.gpsimd.dma_start(
                    out=rec,
                    in_=bass.AP(tensor=rsrc.tensor, offset=rsrc.offset,
                                ap=[[0, 64], [1, 1024]]),
                )
                for h2 in range(2):
                    if qt == 0:
                        o_dst = ot_sb[h2 * 64:h2 * 64 + 64, cc, :]
                    else:
                        o_dst = ot8_sb[h2 * 64:h2 * 64 + 64, cc // 2,
                                       cc % 2, qsl]
                    nc.vector.tensor_mul(o_dst, u_sb[:, h2, :],
                                         rec[:, h2 * 512:(h2 + 1) * 512])
                # leftover independent work lands here
                while fi < len(fillers):
                    fillers[fi]()
                    fi += 1

        # tail: the one remaining output-projection tile
        for oc in range(D // P):
            oproj_chunk(1, oc)()

    nc.finalize()
    return nc


def _make_mask():
    keys = np.arange(4)[None, :, None] * 128 + np.arange(128)[:, None, None]
    qs = np.arange(512)[None, None, :]
    return (keys <= qs).astype(np.float32)


def _to_f8(a):
    return np.clip(a, -240.0, 240.0).astype(NP_F8)


def kernel(X, W_Q, W_K, W_V, W_O):
    global LAST_RESULT, _NC_CACHE
    X = np.asarray(X, dtype=np.float32)
    W_Q = np.asarray(W_Q, dtype=np.float32)
    W_K = np.asarray(W_K, dtype=np.float32)
    W_V = np.asarray(W_V, dtype=np.float32)
    W_O = np.asarray(W_O, dtype=np.float32)

    mask = _make_mask().astype(ml_dtypes.bfloat16)
    in_maps = []
    for c in range(8):
        b, g = c // 2, c % 2
        sl = slice(g * 512, (g + 1) * 512)
        XT = X[b].T  # [d, t]
        # DoubleRow fold: d = k4*256 + i*128 + p -> xt8[p, tt, k4, i, ti]
        xt8 = np.ascontiguousarray(
            XT.reshape(K4, 2, P, TT, 512).transpose(2, 3, 0, 1, 4)
        )
        xt8 = _to_f8(xt8)
        # bf16 X, token tile 0 only (for the bf16 V projection):
        # d = kc*128 + p -> xt0[p, kc, ti]
        xt0 = np.ascontiguousarray(
            XT[:, 0:512].reshape(DIN_C, P, 512).transpose(1, 0, 2)
        ).astype(ml_dtypes.bfloat16)
        # W[sl,:].T is [d_in, c_out]; DoubleRow fold on d_in, c_out split
        # by channel chunk: -> w8[cc, p, k4, i, ci]
        wq8 = _to_f8(np.ascontiguousarray(
            W_Q[sl, :].T.reshape(K4, 2, P, CC, 128).transpose(3, 2, 0, 1, 4)
        ))
        wk8 = _to_f8(np.ascontiguousarray(
            W_K[sl, :].T.reshape(K4, 2, P, CC, 128).transpose(3, 2, 0, 1, 4)
        ))
        # W_V[sl,:].T bf16 -> [p, kc, c] and fp8 DoubleRow -> [p, k4, i, c]
        WVT = W_V[sl, :].T
        wv = np.ascontiguousarray(
            WVT.reshape(DIN_C, P, 512).transpose(1, 0, 2)
        ).astype(ml_dtypes.bfloat16)
        wv8 = _to_f8(np.ascontiguousarray(
            WVT.reshape(K4, 2, P, 512).transpose(2, 0, 1, 3)
        ))
        # W_O[:, sl].T is [c_in(512), o(1024)]; c_in = cc*128 + p -> [p, cc, o]
        WOT = W_O[:, sl].T
        wo = np.ascontiguousarray(
            WOT.reshape(CC, P, D).transpose(1, 0, 2)
        ).astype(ml_dtypes.bfloat16)
        # DoubleRow fold for the fp8 O-projection: c_in = c2*256 + i*128 + p
        wo8 = _to_f8(np.ascontiguousarray(
            WOT.reshape(2, 2, P, D).transpose(2, 0, 1, 3)
        ))
        in_maps.append({
            "ones": np.ones((128, 16 * 8 * 65), dtype=ml_dtypes.bfloat16),
            "ones8": np.ones((128, 8 * 2 * 8 * VH8), dtype=NP_F8),
            "wot8": wo8,
            "xt8": xt8,
            "xt0": xt0,
            "wq8": wq8,
            "wk8": wk8,
            "wvt": wv,
            "wv8": wv8,
            "wot": wo,
            "mask": mask,
        })

    if _NC_CACHE is None:
        _NC_CACHE = build_nc()
    res = run_bass_kernel_spmd(_NC_CACHE, in_maps, core_ids=list(range(8)))
    LAST_RESULT = res

    out = np.empty((B, S, D), dtype=np.float32)
    for b in range(B):
        yt = res.results[2 * b]["yt"] + res.results[2 * b + 1]["yt"]
        out[b] = yt.T
    return out
